# revision 2
# baseline (speedup 1.0000x reference)
"""Block-sparse (block-diagonal local) attention head for Trainium2, 8-way
data-parallel over the batch dimension (one batch element per NeuronCore).

Contract: kernel(**inputs) takes the FULL inputs from setup_inputs() and
returns the FULL output of reference(): out [8, 4096, 128] float32.

Per-core math (batch b):
  kT = (x_b @ Wk)^T, vT = (x_b @ Wv)^T, qT = (x_b @ Wq)^T   (Dh on partitions)
  per 128-token block j:
    v_j   = transpose(vT_j)                    (PE transpose, token-major)
    sT_j  = scoresT[k,q] = sum_d kT[d,k] qT[d,q]
    PT_j  = exp(sT_j / sqrt(Dh))               (no max-subtraction; logits are
                                                O(10) here, softmax algebra is
                                                exact without it)
    o'_j  = PT_j^T @ [v_j | 1 | 1]             (ones columns give row sums)
    out_j = o'_j[:, :128] * (1 / o'_j[:, 128])

Schedule notes (v2, tuned from the NTFF profile of the v1 kernel):
  - ~24 dummy warmup matmuls on zeroed SBUF keep the PE busy from the end of
    the framework preamble so the HAM clock-gate reaches 8/8 (2.4 GHz) by the
    time real data lands (v1 ran its first ~10us of matmuls at 1.2 GHz).
  - DMA priming uses three queues (sync + scalar HWDGE, gpsimd SWDGE) so the
    first x super-tile, wq, wk land as early as possible.  Steady state:
    sync = x chunks 0:4 + output stores, gpsimd = x chunks 4:8, scalar = no
    DMA (keeps ACT free for exp/copies between which a ~0.7us DMA issue
    would stall the attention chain).
  - scores matmuls in bf16 (v1: f32r): 107 -> 57 ns each; q/k staging copies
    stay the same cost (PSUM f32 read bound).
  - 4 v-transposes write one PSUM bank (disjoint column slices of one
    accumulation group) -> one batched DVE copy to SBUF instead of 4.
  - the ones-columns of the v tiles are memset once (persistent tiles), not
    per m-tile.
  - normalization alternates ACT / DVE per block to halve the ACT chain.
  - projections run k, v, q so the tail after the last q matmul is only
    scores -> exp -> o' -> normalize.
"""

import numpy as np
from contextlib import ExitStack

B, S, D, Dh, BLOCK = 8, 4096, 1024, 128, 128
KC = D // 128  # contraction chunks of 128
MT = 512       # token tile (moving free dim of projection matmuls)
STS = 512      # token super-tile per x DMA
NST = S // STS
JT = MT // BLOCK
SCALE = float(1.0 / np.sqrt(np.float32(Dh)))
N_WARMUP = 24  # dummy matmuls to warm the PE clock gate

_CACHE = {}


def _build():
    import concourse.bass as bass
    import concourse.mybir as mybir
    import concourse.tile as tile
    from concourse import bacc

    f32 = mybir.dt.float32
    bf16 = mybir.dt.bfloat16
    ts = bass.ts

    nc = bacc.Bacc("TRN2", target_bir_lowering=False, debug=False)

    xp = nc.dram_tensor("xp", [128, KC, S], bf16, kind="ExternalInput").ap()
    ident_d = nc.dram_tensor("ident", [128, 128], bf16, kind="ExternalInput").ap()
    wp = nc.dram_tensor("wp", [128, 3, KC, Dh], bf16, kind="ExternalInput").ap()
    out = nc.dram_tensor("out", [S, Dh], f32, kind="ExternalOutput").ap()

    with tile.TileContext(nc) as tc, ExitStack() as ctx:
        wpool = ctx.enter_context(tc.tile_pool(name="w", bufs=1))
        cpool = ctx.enter_context(tc.tile_pool(name="const", bufs=1))
        xpool = ctx.enter_context(tc.tile_pool(name="x", bufs=4))
        spool = ctx.enter_context(tc.tile_pool(name="s", bufs=2))
        tpool = ctx.enter_context(tc.tile_pool(name="t", bufs=2))
        opool = ctx.enter_context(tc.tile_pool(name="o", bufs=2))
        rpool = ctx.enter_context(tc.tile_pool(name="r", bufs=4))
        ppool = ctx.enter_context(tc.tile_pool(name="proj_ps", bufs=3, space="PSUM"))
        spsum = ctx.enter_context(tc.tile_pool(name="s_ps", bufs=1, space="PSUM"))
        vpsum = ctx.enter_context(tc.tile_pool(name="v_ps", bufs=1, space="PSUM"))
        qpool = ctx.enter_context(tc.tile_pool(name="o_ps", bufs=3, space="PSUM"))

        # --- PE warmup: zero a small tile, then stream dummy matmuls so the
        # HAM clock-gate sees a busy PE while DMAs prime.  The dummy outputs
        # rotate through the projection PSUM slots and are never read.
        dummy = cpool.tile([128, 128], bf16, tag="dummy")
        nc.vector.memset(dummy[:], 0.0)
        for _ in range(N_WARMUP):
            d_ps = ppool.tile([128, MT], f32, tag="proj")
            nc.tensor.matmul(d_ps[:, 0:128], dummy[:], dummy[:], start=True, stop=True)

        # --- persistent tiles: identity for PE transposes; token-major v
        # tiles ([v | 1 1] per block) whose ones-columns are written once.
        ident = cpool.tile([128, 128], bf16, tag="ident")
        v_mts = []
        for p in range(2):
            v_mt = cpool.tile([128, JT, BLOCK + 2], bf16, tag=f"vmt{p}")
            nc.vector.memset(v_mt[:, :, BLOCK : BLOCK + 2], 1.0)
            v_mts.append(v_mt)

        # --- DMA priming (st == 0) across three queues:
        #   scalar: wq chunk0 (tiny; gates the first matmul), wq rest,
        #           x[4:8] for st 0 and 1, ident
        #   sync:   x[0:1], x[1:2], x[2:4], then x[0:4] per super-tile + stores
        #   gpsimd: wk, wv, then x[4:8] for st >= 2
        wp_t = wpool.tile([128, 3, KC, Dh], bf16, tag="wp")
        nc.scalar.dma_start(wp_t[:, 0:1, 0:1], wp[:, 0:1, 0:1])
        nc.scalar.dma_start(wp_t[:, 0:1, 1:KC], wp[:, 0:1, 1:KC])
        nc.gpsimd.dma_start(wp_t[:, 1:2], wp[:, 1:2])  # wk
        nc.gpsimd.dma_start(wp_t[:, 2:3], wp[:, 2:3])  # wv

        for st in range(NST):
            s0 = st * STS
            xt = xpool.tile([128, KC, STS], bf16, tag="xt")
            if st == 0:
                nc.sync.dma_start(xt[:, 0:1], xp[:, 0:1, s0 : s0 + STS])
                nc.sync.dma_start(xt[:, 1:2], xp[:, 1:2, s0 : s0 + STS])
                nc.sync.dma_start(xt[:, 2:4], xp[:, 2:4, s0 : s0 + STS])
                nc.scalar.dma_start(xt[:, 4:KC], xp[:, 4:KC, s0 : s0 + STS])
                nc.scalar.dma_start(ident[:], ident_d[:])
            elif st == 1:
                nc.sync.dma_start(xt[:, 0:4], xp[:, 0:4, s0 : s0 + STS])
                nc.scalar.dma_start(xt[:, 4:KC], xp[:, 4:KC, s0 : s0 + STS])
            else:
                nc.sync.dma_start(xt[:, 0:4], xp[:, 0:4, s0 : s0 + STS])
                nc.gpsimd.dma_start(xt[:, 4:KC], xp[:, 4:KC, s0 : s0 + STS])

            for sub in range(STS // MT):
                moff = sub * MT
                m0 = s0 + moff
                last = st == NST - 1

                # Projections (Dh on partitions): pT[d, m] = sum_k W[k,d] x[k,m]
                # Order k, v, q: the transposes/copies of v and the kT staging
                # overlap the q projection, so after the last q matmul only
                # scores -> exp -> o' remain.
                pT_sbs = {}
                for wi, tag, copy_eng in (
                    (1, "kT", nc.scalar),
                    (2, "vT", nc.vector),
                    (0, "qT", nc.vector),
                ):
                    pT_ps = ppool.tile([128, MT], f32, tag="proj")
                    for k in range(KC):
                        nc.tensor.matmul(
                            pT_ps[:],
                            wp_t[:, wi, k, :],
                            xt[:, k, moff : moff + MT],
                            start=(k == 0),
                            stop=(k == KC - 1),
                        )
                    pT_sb = spool.tile([128, MT], bf16, tag=tag)
                    if copy_eng is nc.scalar:
                        nc.scalar.copy(pT_sb[:], pT_ps[:])
                    else:
                        nc.vector.tensor_copy(pT_sb[:], pT_ps[:])
                    pT_sbs[tag] = pT_sb

                    if tag == "vT":
                        # 4 PE transposes into one PSUM bank (disjoint column
                        # slices of one accumulation group), then one batched
                        # copy into the persistent [v | 1 1] tile.
                        v_ps = vpsum.tile([128, JT, BLOCK], bf16, tag="vps")
                        for j in range(JT):
                            nc.tensor.matmul(
                                v_ps[:, j],
                                pT_sb[:, ts(j, BLOCK)],
                                ident[:],
                                is_transpose=True,
                                start=(j == 0),
                                stop=(j == JT - 1),
                            )
                        v_mt = v_mts[(st * (STS // MT) + sub) % 2]
                        nc.vector.tensor_copy(v_mt[:, :, 0:BLOCK], v_ps[:])
                qT_sb, kT_sb = pT_sbs["qT"], pT_sbs["kT"]

                # All JT blocks' scoresT into one PSUM bank, bf16 operands.
                sT_big = spsum.tile([128, JT * BLOCK], f32, tag="sT")
                for j in range(JT):
                    blk = ts(j, BLOCK)
                    nc.tensor.matmul(
                        sT_big[:, blk],
                        kT_sb[:, blk],
                        qT_sb[:, blk],
                        start=(j == 0),
                        stop=(j == JT - 1),
                    )
                PT_big = tpool.tile([128, JT * BLOCK], bf16, tag="PT")
                if last:
                    # no projection work left to hide the exp latency: split
                    # so the first o' matmuls start after half the exp
                    H = JT * BLOCK // 2
                    nc.scalar.activation(
                        PT_big[:, 0:H], sT_big[:, 0:H],
                        mybir.ActivationFunctionType.Exp, scale=SCALE,
                    )
                    nc.scalar.activation(
                        PT_big[:, H:], sT_big[:, H:],
                        mybir.ActivationFunctionType.Exp, scale=SCALE,
                    )
                else:
                    nc.scalar.activation(
                        PT_big[:], sT_big[:],
                        mybir.ActivationFunctionType.Exp, scale=SCALE,
                    )

                o_mt = opool.tile([128, JT, BLOCK], f32, tag="o_mt")
                for j in range(JT):
                    blk = ts(j, BLOCK)
                    o_ps = qpool.tile([128, BLOCK + 2], f32, tag="ops")
                    nc.tensor.matmul(
                        o_ps[:], PT_big[:, blk], v_mt[:, j], start=True, stop=True
                    )
                    r_sb = rpool.tile([128, 1], f32, tag="r")
                    nc.vector.reciprocal(r_sb[:], o_ps[:, BLOCK : BLOCK + 1])
                    # normalize: alternate ACT / DVE so neither engine's queue
                    # serializes the whole m-tile
                    if j % 2 == 0:
                        nc.scalar.activation(
                            o_mt[:, j], o_ps[:, 0:BLOCK],
                            mybir.ActivationFunctionType.Copy, scale=r_sb[:],
                        )
                    else:
                        nc.vector.tensor_scalar_mul(
                            o_mt[:, j], o_ps[:, 0:BLOCK], r_sb[:]
                        )

                # out[m0 + c*BLOCK + p, d] <- o_mt[p, c, d]
                out_view = out[m0 : m0 + MT, :].rearrange(
                    "(c p) d -> p c d", p=BLOCK
                )
                if last:
                    # final stores split across both HWDGE rings
                    nc.sync.dma_start(out_view[:, 0:2], o_mt[:, 0:2])
                    nc.scalar.dma_start(out_view[:, 2:4], o_mt[:, 2:4])
                else:
                    nc.sync.dma_start(out_view, o_mt[:])

    nc.compile()
    return nc


def _get_nc():
    if "nc" not in _CACHE:
        _CACHE["nc"] = _build()
    return _CACHE["nc"]


def make_in_maps(x, Wq, Wk, Wv):
    import ml_dtypes

    proj_np = ml_dtypes.bfloat16
    # wp[p, i, k, d] = W_i[k*128 + p, d]
    wp = np.stack(
        [np.asarray(w).reshape(KC, 128, Dh).transpose(1, 0, 2) for w in (Wq, Wk, Wv)],
        axis=1,
    )
    wp_h = np.ascontiguousarray(wp.astype(proj_np))
    ident_h = np.eye(128, dtype=proj_np)
    x = np.asarray(x)
    maps = []
    for b in range(B):
        # xp[p, k, s] = x[b].T[k*128 + p, s]
        xp = np.asarray(x[b], dtype=proj_np).T.reshape(KC, 128, S).transpose(1, 0, 2)
        maps.append(
            {
                "xp": np.ascontiguousarray(xp),
                "wp": wp_h,
                "ident": ident_h,
            }
        )
    return maps


def kernel(x, Wq, Wk, Wv):
    from concourse.bass_utils import run_bass_kernel_spmd

    nc = _get_nc()
    in_maps = make_in_maps(x, Wq, Wk, Wv)
    res = run_bass_kernel_spmd(nc, in_maps, list(range(B))).results
    return np.stack([res[b]["out"] for b in range(B)], axis=0)


# revision 3
# speedup vs baseline: 1.0103x; 1.0103x over previous
"""Block-sparse (block-diagonal local) attention head for Trainium2, 8-way
data-parallel over the batch dimension (one batch element per NeuronCore).

Contract: kernel(**inputs) takes the FULL inputs from setup_inputs() and
returns the FULL output of reference(): out [8, 4096, 128] float32.

Per-core math (batch b):
  kT = (x_b @ Wk)^T, vT = (x_b @ Wv)^T, qT = (x_b @ Wq)^T   (Dh on partitions)
  per 128-token block j:
    v_j   = transpose(vT_j)                    (PE transpose, token-major)
    sT_j  = scoresT[k,q] = sum_d kT[d,k] qT[d,q]
    PT_j  = exp(sT_j / sqrt(Dh))               (no max-subtraction; logits are
                                                O(10) here, softmax algebra is
                                                exact without it)
    o'_j  = PT_j^T @ [v_j | 1 | 1]             (ones columns give row sums)
    out_j = o'_j[:, :128] * (1 / o'_j[:, 128])

Schedule notes (v3, tuned from NTFF profiles of v1/v2):
  - ~20 dummy warmup matmuls on zeroed SBUF keep the PE busy from the end of
    the framework preamble so the HAM clock-gate reaches 8/8 (2.4 GHz) by the
    time real data lands (v1 ran its first ~10us of matmuls at 1.2 GHz).
  - The first super-tile runs K-MAJOR (q,k,v interleaved per k-chunk): 3x the
    matmul work per arrived x byte, matching the warm PE consumption rate to
    the HBM supply rate.  v2 ran q,k,v sequentially, outran the DMA by 2x,
    starved, and the HAM re-throttled mid-ramp (10us at half clock).
  - DMA priming over three queues (sync + scalar HWDGE, gpsimd SWDGE), with
    one combined [wq|wk|wv] k=0 chunk transfer gating the first real matmuls.
    Steady state: sync = x[0:4] + stores, gpsimd = x[4:8], scalar = no DMA.
  - scores matmuls in bf16 (v1: f32r): 107 -> 57 ns each.
  - 4 v-transposes write one PSUM bank -> one batched DVE copy out; the
    ones-columns of the [v | 1 1] tiles are persistent (memset once).
  - normalization alternates ACT / DVE per block.
  - last m-tile: q-projection split into two N=256 groups in different PSUM
    banks, so qT evacuation + scores + exp of the first half overlap the
    second half's matmuls; o'_3 reuses the scores bank; one store per block.
"""

import numpy as np
from contextlib import ExitStack

B, S, D, Dh, BLOCK = 8, 4096, 1024, 128, 128
KC = D // 128  # contraction chunks of 128
MT = 512       # token tile (moving free dim of projection matmuls)
STS = 512      # token super-tile per x DMA
NST = S // STS
JT = MT // BLOCK
SCALE = float(1.0 / np.sqrt(np.float32(Dh)))
N_WARMUP = 20  # dummy matmuls to warm the PE clock gate

_CACHE = {}


def _build():
    import concourse.bass as bass
    import concourse.mybir as mybir
    import concourse.tile as tile
    from concourse import bacc

    f32 = mybir.dt.float32
    bf16 = mybir.dt.bfloat16
    ts = bass.ts
    Exp = mybir.ActivationFunctionType.Exp
    Copy = mybir.ActivationFunctionType.Copy

    nc = bacc.Bacc("TRN2", target_bir_lowering=False, debug=False)

    xp = nc.dram_tensor("xp", [128, KC, S], bf16, kind="ExternalInput").ap()
    ident_d = nc.dram_tensor("ident", [128, 128], bf16, kind="ExternalInput").ap()
    wp = nc.dram_tensor("wp", [128, 3, KC, Dh], bf16, kind="ExternalInput").ap()
    out = nc.dram_tensor("out", [S, Dh], f32, kind="ExternalOutput").ap()

    with tile.TileContext(nc) as tc, ExitStack() as ctx:
        wpool = ctx.enter_context(tc.tile_pool(name="w", bufs=1))
        cpool = ctx.enter_context(tc.tile_pool(name="const", bufs=1))
        xpool = ctx.enter_context(tc.tile_pool(name="x", bufs=4))
        spool = ctx.enter_context(tc.tile_pool(name="s", bufs=2))
        tpool = ctx.enter_context(tc.tile_pool(name="t", bufs=2))
        opool = ctx.enter_context(tc.tile_pool(name="o", bufs=2))
        rpool = ctx.enter_context(tc.tile_pool(name="r", bufs=4))
        ppool = ctx.enter_context(tc.tile_pool(name="proj_ps", bufs=3, space="PSUM"))
        spsum = ctx.enter_context(tc.tile_pool(name="s_ps", bufs=1, space="PSUM"))
        vpsum = ctx.enter_context(tc.tile_pool(name="v_ps", bufs=1, space="PSUM"))
        qpool = ctx.enter_context(tc.tile_pool(name="o_ps", bufs=3, space="PSUM"))

        # --- PE warmup: zero a small tile, then stream dummy matmuls so the
        # HAM clock-gate sees a busy PE while the DMA rings prime.
        dummy = cpool.tile([128, 128], bf16, tag="dummy")
        nc.vector.memset(dummy[:], 0.0)
        for _ in range(N_WARMUP):
            d_ps = ppool.tile([128, MT], f32, tag="proj")
            nc.tensor.matmul(d_ps[:, 0:128], dummy[:], dummy[:], start=True, stop=True)

        # --- persistent tiles
        ident = cpool.tile([128, 128], bf16, tag="ident")
        v_mts = []
        for p in range(2):
            v_mt = cpool.tile([128, JT, BLOCK + 2], bf16, tag=f"vmt{p}")
            nc.vector.memset(v_mt[:, :, BLOCK : BLOCK + 2], 1.0)
            v_mts.append(v_mt)

        # --- DMA priming.  Arrival order is matched to the k-major
        # consumption order of super-tile 0 (see header).
        wp_t = wpool.tile([128, 3, KC, Dh], bf16, tag="wp")
        # one combined [wq|wk|wv] k=0 chunk: gates the first three matmuls
        nc.scalar.dma_start(wp_t[:, :, 0:1], wp[:, :, 0:1])
        nc.scalar.dma_start(wp_t[:, 1:2, 1:KC], wp[:, 1:2, 1:KC])  # wk rest
        nc.gpsimd.dma_start(wp_t[:, 2:3, 1:KC], wp[:, 2:3, 1:KC])  # wv rest

        for st in range(NST):
            s0 = st * STS
            xt = xpool.tile([128, KC, STS], bf16, tag="xt")
            if st == 0:
                nc.sync.dma_start(xt[:, 0:1], xp[:, 0:1, s0 : s0 + STS])
                nc.sync.dma_start(wp_t[:, 0:1, 1:KC], wp[:, 0:1, 1:KC])  # wq rest
                nc.sync.dma_start(xt[:, 1:2], xp[:, 1:2, s0 : s0 + STS])
                nc.sync.dma_start(xt[:, 2:4], xp[:, 2:4, s0 : s0 + STS])
                nc.gpsimd.dma_start(xt[:, 4:6], xp[:, 4:6, s0 : s0 + STS])
                nc.gpsimd.dma_start(xt[:, 6:KC], xp[:, 6:KC, s0 : s0 + STS])
                nc.scalar.dma_start(ident[:], ident_d[:])
            elif st == 1:
                nc.sync.dma_start(xt[:, 0:4], xp[:, 0:4, s0 : s0 + STS])
                nc.scalar.dma_start(xt[:, 4:KC], xp[:, 4:KC, s0 : s0 + STS])
            else:
                nc.sync.dma_start(xt[:, 0:4], xp[:, 0:4, s0 : s0 + STS])
                nc.gpsimd.dma_start(xt[:, 4:KC], xp[:, 4:KC, s0 : s0 + STS])

            for sub in range(STS // MT):
                moff = sub * MT
                m0 = s0 + moff
                last = st == NST - 1 and sub == STS // MT - 1
                mi = st * (STS // MT) + sub

                # ---- projections: pT[d, m] = sum_k W[k,d] xT[k,m]
                qT_sb = spool.tile([128, MT], bf16, tag="qT")
                kT_sb = spool.tile([128, MT], bf16, tag="kT")
                vT_sb = spool.tile([128, MT], bf16, tag="vT")
                v_mt = v_mts[mi % 2]

                def v_transposes(vT_sb=vT_sb, v_mt=v_mt):
                    # 4 PE transposes into one PSUM bank (disjoint column
                    # slices of one accumulation group) + one batched copy
                    # into the persistent [v | 1 1] tile.
                    v_ps = vpsum.tile([128, JT, BLOCK], bf16, tag="vps")
                    for j in range(JT):
                        nc.tensor.matmul(
                            v_ps[:, j],
                            vT_sb[:, ts(j, BLOCK)],
                            ident[:],
                            is_transpose=True,
                            start=(j == 0),
                            stop=(j == JT - 1),
                        )
                    nc.vector.tensor_copy(v_mt[:, :, 0:BLOCK], v_ps[:])

                if st == 0:
                    # k-major: q,k,v per k-chunk so each arrived x chunk
                    # feeds 3 matmuls (supply-rate matched ramp)
                    q_ps = ppool.tile([128, MT], f32, tag="proj")
                    k_ps = ppool.tile([128, MT], f32, tag="proj")
                    v_ps2 = ppool.tile([128, MT], f32, tag="proj")
                    for k in range(KC):
                        for wi, pps in ((0, q_ps), (1, k_ps), (2, v_ps2)):
                            nc.tensor.matmul(
                                pps[:],
                                wp_t[:, wi, k, :],
                                xt[:, k, moff : moff + MT],
                                start=(k == 0),
                                stop=(k == KC - 1),
                            )
                    nc.scalar.copy(kT_sb[:], k_ps[:])
                    nc.vector.tensor_copy(vT_sb[:], v_ps2[:])
                    v_transposes()
                    nc.vector.tensor_copy(qT_sb[:], q_ps[:])
                else:
                    # sequential k, v, q: v's transposes/copies and kT
                    # staging overlap the q projection
                    for wi, sb, copy_eng in (
                        (1, kT_sb, nc.scalar),
                        (2, vT_sb, nc.vector),
                        (0, qT_sb, None),
                    ):
                        if wi == 0 and last:
                            break
                        pps = ppool.tile([128, MT], f32, tag="proj")
                        for k in range(KC):
                            nc.tensor.matmul(
                                pps[:],
                                wp_t[:, wi, k, :],
                                xt[:, k, moff : moff + MT],
                                start=(k == 0),
                                stop=(k == KC - 1),
                            )
                        if copy_eng is nc.scalar:
                            nc.scalar.copy(sb[:], pps[:])
                        else:
                            nc.vector.tensor_copy(sb[:], pps[:])
                        if wi == 2:
                            v_transposes()
                    if last:
                        # split q into two N=256 groups in different banks:
                        # first half's evacuation + scores + exp overlap the
                        # second half's matmuls
                        H = MT // 2
                        qA = ppool.tile([128, H], f32, tag="proj")
                        for k in range(KC):
                            nc.tensor.matmul(
                                qA[:],
                                wp_t[:, 0, k, :],
                                xt[:, k, moff : moff + H],
                                start=(k == 0),
                                stop=(k == KC - 1),
                            )
                        nc.vector.tensor_copy(qT_sb[:, 0:H], qA[:])
                        qB = vpsum.tile([128, H], f32, tag="vps")
                        for k in range(KC):
                            nc.tensor.matmul(
                                qB[:],
                                wp_t[:, 0, k, :],
                                xt[:, k, moff + H : moff + MT],
                                start=(k == 0),
                                stop=(k == KC - 1),
                            )
                        nc.scalar.copy(qT_sb[:, H:MT], qB[:])

                # ---- attention
                sT_big = spsum.tile([128, JT * BLOCK], f32, tag="sT")
                for j in range(JT):
                    blk = ts(j, BLOCK)
                    nc.tensor.matmul(
                        sT_big[:, blk],
                        kT_sb[:, blk],
                        qT_sb[:, blk],
                        start=(j == 0),
                        stop=(j == JT - 1),
                    )
                PT_big = tpool.tile([128, JT * BLOCK], bf16, tag="PT")
                if last:
                    H = JT * BLOCK // 2
                    nc.scalar.activation(PT_big[:, 0:H], sT_big[:, 0:H], Exp, scale=SCALE)
                    nc.scalar.activation(PT_big[:, H:], sT_big[:, H:], Exp, scale=SCALE)
                else:
                    nc.scalar.activation(PT_big[:], sT_big[:], Exp, scale=SCALE)

                o_mt = opool.tile([128, JT, BLOCK], f32, tag="o_mt")
                out_view = out[m0 : m0 + MT, :].rearrange("(c p) d -> p c d", p=BLOCK)
                for j in range(JT):
                    blk = ts(j, BLOCK)
                    if last and j == JT - 1:
                        # the scores bank is free once exp has read it
                        o_ps = spsum.tile([128, BLOCK + 2], f32, tag="sT")
                    else:
                        o_ps = qpool.tile([128, BLOCK + 2], f32, tag="ops")
                    nc.tensor.matmul(
                        o_ps[:], PT_big[:, blk], v_mt[:, j], start=True, stop=True
                    )
                    r_sb = rpool.tile([128, 1], f32, tag="r")
                    nc.vector.reciprocal(r_sb[:], o_ps[:, BLOCK : BLOCK + 1])
                    # normalize: alternate engines; at the tail ACT is busy
                    # with the exps, so give DVE the even blocks there
                    on_act = (j % 2 == 1) if last else (j % 2 == 0)
                    if on_act:
                        nc.scalar.activation(
                            o_mt[:, j], o_ps[:, 0:BLOCK], Copy, scale=r_sb[:]
                        )
                    else:
                        nc.vector.tensor_scalar_mul(o_mt[:, j], o_ps[:, 0:BLOCK], r_sb[:])
                    if last:
                        # store per block as it completes, alternating rings
                        eng = nc.sync if j % 2 == 0 else nc.scalar
                        eng.dma_start(out_view[:, j : j + 1], o_mt[:, j : j + 1])

                if not last:
                    nc.sync.dma_start(out_view, o_mt[:])

    nc.compile()
    return nc


def _get_nc():
    if "nc" not in _CACHE:
        _CACHE["nc"] = _build()
    return _CACHE["nc"]


def make_in_maps(x, Wq, Wk, Wv):
    import ml_dtypes

    proj_np = ml_dtypes.bfloat16
    # wp[p, i, k, d] = W_i[k*128 + p, d]
    wp = np.stack(
        [np.asarray(w).reshape(KC, 128, Dh).transpose(1, 0, 2) for w in (Wq, Wk, Wv)],
        axis=1,
    )
    wp_h = np.ascontiguousarray(wp.astype(proj_np))
    ident_h = np.eye(128, dtype=proj_np)
    x = np.asarray(x)
    maps = []
    for b in range(B):
        # xp[p, k, s] = x[b].T[k*128 + p, s]
        xp = np.asarray(x[b], dtype=proj_np).T.reshape(KC, 128, S).transpose(1, 0, 2)
        maps.append(
            {
                "xp": np.ascontiguousarray(xp),
                "wp": wp_h,
                "ident": ident_h,
            }
        )
    return maps


def kernel(x, Wq, Wk, Wv):
    from concourse.bass_utils import run_bass_kernel_spmd

    nc = _get_nc()
    in_maps = make_in_maps(x, Wq, Wk, Wv)
    res = run_bass_kernel_spmd(nc, in_maps, list(range(B))).results
    return np.stack([res[b]["out"] for b in range(B)], axis=0)


# revision 7
# speedup vs baseline: 1.0578x; 1.0470x over previous
"""Block-sparse (block-diagonal local) attention head for Trainium2, 8-way
data-parallel over the batch dimension (one batch element per NeuronCore).

Contract: kernel(**inputs) takes the FULL inputs from setup_inputs() and
returns the FULL output of reference(): out [8, 4096, 128] float32.

Per-core math (batch b):
  kT = (x_b @ Wk)^T, vT = (x_b @ Wv)^T, qT = (x_b @ Wq)^T   (Dh on partitions)
  per 128-token block j:
    v_j   = transpose(vT_j)                    (PE transpose, token-major)
    sT_j  = scoresT[k,q] = sum_d kT[d,k] qT[d,q]
    PT_j  = exp(sT_j / sqrt(Dh))               (no max-subtraction; logits are
                                                O(10) here, softmax algebra is
                                                exact without it)
    o'_j  = PT_j^T @ [v_j | 1 | 1]             (ones columns give row sums)
    out_j = o'_j[:, :128] * (1 / o'_j[:, 128])

Schedule notes (v3, tuned from NTFF profiles of v1/v2):
  - ~20 dummy warmup matmuls on zeroed SBUF keep the PE busy from the end of
    the framework preamble so the HAM clock-gate reaches 8/8 (2.4 GHz) by the
    time real data lands (v1 ran its first ~10us of matmuls at 1.2 GHz).
  - The first super-tile runs K-MAJOR (q,k,v interleaved per k-chunk): 3x the
    matmul work per arrived x byte, matching the warm PE consumption rate to
    the HBM supply rate.  v2 ran q,k,v sequentially, outran the DMA by 2x,
    starved, and the HAM re-throttled mid-ramp (10us at half clock).
  - DMA priming over three queues (sync + scalar HWDGE, gpsimd SWDGE), with
    one combined [wq|wk|wv] k=0 chunk transfer gating the first real matmuls.
    Steady state: sync = x[0:4] + stores, gpsimd = x[4:8], scalar = no DMA.
  - scores matmuls in bf16 (v1: f32r): 107 -> 57 ns each.
  - 4 v-transposes write one PSUM bank -> one batched DVE copy out; the
    ones-columns of the [v | 1 1] tiles are persistent (memset once).
  - normalization alternates ACT / DVE per block.
  - last m-tile: q-projection split into two N=256 groups in different PSUM
    banks, so qT evacuation + scores + exp of the first half overlap the
    second half's matmuls; o'_3 reuses the scores bank; one store per block.
"""

import numpy as np
from contextlib import ExitStack

B, S, D, Dh, BLOCK = 8, 4096, 1024, 128, 128
KC = D // 128  # contraction chunks of 128
MT = 512       # token tile (moving free dim of projection matmuls)
STS = 512      # token super-tile per x DMA
NST = S // STS
JT = MT // BLOCK
SCALE = float(1.0 / np.sqrt(np.float32(Dh)))
N_WARMUP = 20  # dummy matmuls to warm the PE clock gate

_CACHE = {}


def _build():
    import concourse.bass as bass
    import concourse.mybir as mybir
    import concourse.tile as tile
    from concourse import bacc

    f32 = mybir.dt.float32
    bf16 = mybir.dt.bfloat16
    ts = bass.ts
    Exp = mybir.ActivationFunctionType.Exp
    Copy = mybir.ActivationFunctionType.Copy

    nc = bacc.Bacc("TRN2", target_bir_lowering=False, debug=False)

    xp = nc.dram_tensor("xp", [128, KC, S], bf16, kind="ExternalInput").ap()
    ident_d = nc.dram_tensor("ident", [128, 128], bf16, kind="ExternalInput").ap()
    wp = nc.dram_tensor("wp", [128, 3, KC, Dh], bf16, kind="ExternalInput").ap()
    out = nc.dram_tensor("out", [S, Dh], f32, kind="ExternalOutput").ap()

    with tile.TileContext(nc) as tc, ExitStack() as ctx:
        wpool = ctx.enter_context(tc.tile_pool(name="w", bufs=1))
        cpool = ctx.enter_context(tc.tile_pool(name="const", bufs=1))
        xpool = ctx.enter_context(tc.tile_pool(name="x", bufs=4))
        spool = ctx.enter_context(tc.tile_pool(name="s", bufs=2))
        tpool = ctx.enter_context(tc.tile_pool(name="t", bufs=2))
        opool = ctx.enter_context(tc.tile_pool(name="o", bufs=2))
        rpool = ctx.enter_context(tc.tile_pool(name="r", bufs=4))
        ppool = ctx.enter_context(tc.tile_pool(name="proj_ps", bufs=3, space="PSUM"))
        spsum = ctx.enter_context(tc.tile_pool(name="s_ps", bufs=1, space="PSUM"))
        vpsum = ctx.enter_context(tc.tile_pool(name="v_ps", bufs=1, space="PSUM"))
        qpool = ctx.enter_context(tc.tile_pool(name="o_ps", bufs=3, space="PSUM"))

        # --- PE warmup: zero a small tile, then stream dummy matmuls so the
        # HAM clock-gate sees a busy PE while the DMA rings prime.
        dummy = cpool.tile([128, 128], bf16, tag="dummy")
        nc.vector.memset(dummy[:], 0.0)
        for _ in range(N_WARMUP):
            d_ps = ppool.tile([128, MT], f32, tag="proj")
            nc.tensor.matmul(d_ps[:, 0:128], dummy[:], dummy[:], start=True, stop=True)

        # --- persistent tiles
        ident = cpool.tile([128, 128], bf16, tag="ident")
        v_mts = []
        for p in range(2):
            v_mt = cpool.tile([128, JT, BLOCK + 2], bf16, tag=f"vmt{p}")
            nc.vector.memset(v_mt[:, :, BLOCK : BLOCK + 2], 1.0)
            v_mts.append(v_mt)

        # --- DMA priming.  All x loads ride the sync HWDGE ring in exact
        # consumption order (one FIFO ring -> arrival order == issue order;
        # multi-ring splits let the SDMA round-robin starve the critical
        # chunk behind bulk traffic).  Weights arrive as per-k combined
        # [wq|wk|wv] chunks on scalar, matching the k-major consumption of
        # super-tile 0.  Mid-kernel output stores go to gpsimd (SWDGE) so
        # neither HWDGE ring nor the ACT queue is disturbed.
        wp_t = wpool.tile([128, 3, KC, Dh], bf16, tag="wp")
        for k in range(KC):
            nc.scalar.dma_start(wp_t[:, :, k : k + 1], wp[:, :, k : k + 1])
        nc.scalar.dma_start(ident[:], ident_d[:])

        for st in range(NST):
            s0 = st * STS
            xt = xpool.tile([128, KC, STS], bf16, tag="xt")
            if st == 0:
                nc.sync.dma_start(xt[:, 0:1], xp[:, 0:1, s0 : s0 + STS])
                nc.sync.dma_start(xt[:, 1:2], xp[:, 1:2, s0 : s0 + STS])
                nc.sync.dma_start(xt[:, 2:4], xp[:, 2:4, s0 : s0 + STS])
                nc.sync.dma_start(xt[:, 4:6], xp[:, 4:6, s0 : s0 + STS])
                nc.sync.dma_start(xt[:, 6:KC], xp[:, 6:KC, s0 : s0 + STS])
            else:
                nc.sync.dma_start(xt[:, 0:4], xp[:, 0:4, s0 : s0 + STS])
                nc.sync.dma_start(xt[:, 4:KC], xp[:, 4:KC, s0 : s0 + STS])

            for sub in range(STS // MT):
                moff = sub * MT
                m0 = s0 + moff
                last = st == NST - 1 and sub == STS // MT - 1
                mi = st * (STS // MT) + sub

                # ---- projections: pT[d, m] = sum_k W[k,d] xT[k,m]
                qT_sb = spool.tile([128, MT], bf16, tag="qT")
                kT_sb = spool.tile([128, MT], bf16, tag="kT")
                vT_sb = spool.tile([128, MT], bf16, tag="vT")
                v_mt = v_mts[mi % 2]

                def v_transposes(vT_sb=vT_sb, v_mt=v_mt):
                    # 4 PE transposes into one PSUM bank (disjoint column
                    # slices of one accumulation group) + one batched copy
                    # into the persistent [v | 1 1] tile.
                    v_ps = vpsum.tile([128, JT, BLOCK], bf16, tag="vps")
                    for j in range(JT):
                        nc.tensor.matmul(
                            v_ps[:, j],
                            vT_sb[:, ts(j, BLOCK)],
                            ident[:],
                            is_transpose=True,
                            start=(j == 0),
                            stop=(j == JT - 1),
                        )
                    nc.vector.tensor_copy(v_mt[:, :, 0:BLOCK], v_ps[:])

                if st == 0:
                    # k-major: q,k,v per k-chunk so each arrived x chunk
                    # feeds 3 matmuls (supply-rate matched ramp)
                    q_ps = ppool.tile([128, MT], f32, tag="proj")
                    k_ps = ppool.tile([128, MT], f32, tag="proj")
                    v_ps2 = ppool.tile([128, MT], f32, tag="proj")
                    for k in range(KC):
                        for wi, pps in ((0, q_ps), (1, k_ps), (2, v_ps2)):
                            nc.tensor.matmul(
                                pps[:],
                                wp_t[:, wi, k, :],
                                xt[:, k, moff : moff + MT],
                                start=(k == 0),
                                stop=(k == KC - 1),
                            )
                    nc.scalar.copy(kT_sb[:], k_ps[:])
                    nc.vector.tensor_copy(vT_sb[:], v_ps2[:])
                    v_transposes()
                    nc.vector.tensor_copy(qT_sb[:], q_ps[:])
                else:
                    # sequential k, v, q: v's transposes/copies and kT
                    # staging overlap the q projection
                    for wi, sb, copy_eng in (
                        (1, kT_sb, nc.scalar),
                        (2, vT_sb, nc.vector),
                        (0, qT_sb, None),
                    ):
                        if wi == 0 and last:
                            break
                        pps = ppool.tile([128, MT], f32, tag="proj")
                        for k in range(KC):
                            nc.tensor.matmul(
                                pps[:],
                                wp_t[:, wi, k, :],
                                xt[:, k, moff : moff + MT],
                                start=(k == 0),
                                stop=(k == KC - 1),
                            )
                        if copy_eng is nc.scalar:
                            nc.scalar.copy(sb[:], pps[:])
                        else:
                            nc.vector.tensor_copy(sb[:], pps[:])
                        if wi == 2:
                            v_transposes()
                    if last:
                        # split q into two N=256 groups in different banks:
                        # first half's evacuation + scores + exp overlap the
                        # second half's matmuls
                        H = MT // 2
                        qA = ppool.tile([128, H], f32, tag="proj")
                        for k in range(KC):
                            nc.tensor.matmul(
                                qA[:],
                                wp_t[:, 0, k, :],
                                xt[:, k, moff : moff + H],
                                start=(k == 0),
                                stop=(k == KC - 1),
                            )
                        nc.vector.tensor_copy(qT_sb[:, 0:H], qA[:])
                        qB = vpsum.tile([128, H], f32, tag="vps")
                        for k in range(KC):
                            nc.tensor.matmul(
                                qB[:],
                                wp_t[:, 0, k, :],
                                xt[:, k, moff + H : moff + MT],
                                start=(k == 0),
                                stop=(k == KC - 1),
                            )
                        # on DVE: a scalar.copy here would head-of-line block
                        # the exps behind it in the ACT FIFO
                        nc.vector.tensor_copy(qT_sb[:, H:MT], qB[:])

                # ---- attention
                sT_big = spsum.tile([128, JT * BLOCK], f32, tag="sT")
                for j in range(JT):
                    blk = ts(j, BLOCK)
                    nc.tensor.matmul(
                        sT_big[:, blk],
                        kT_sb[:, blk],
                        qT_sb[:, blk],
                        start=(j == 0),
                        stop=(j == JT - 1),
                    )
                PT_big = tpool.tile([128, JT * BLOCK], bf16, tag="PT")
                if last:
                    # per-block exps: each o'_j is gated on 1/4 of the exp
                    for j in range(JT):
                        blk = ts(j, BLOCK)
                        nc.scalar.activation(
                            PT_big[:, blk], sT_big[:, blk], Exp, scale=SCALE
                        )
                else:
                    nc.scalar.activation(PT_big[:], sT_big[:], Exp, scale=SCALE)

                o_mt = opool.tile([128, JT, BLOCK], f32, tag="o_mt")
                out_view = out[m0 : m0 + MT, :].rearrange("(c p) d -> p c d", p=BLOCK)
                for j in range(JT):
                    blk = ts(j, BLOCK)
                    if last and j == JT - 1:
                        # the scores bank is free once exp has read it
                        o_ps = spsum.tile([128, BLOCK + 2], f32, tag="sT")
                    else:
                        o_ps = qpool.tile([128, BLOCK + 2], f32, tag="ops")
                    nc.tensor.matmul(
                        o_ps[:], PT_big[:, blk], v_mt[:, j], start=True, stop=True
                    )
                    r_sb = rpool.tile([128, 1], f32, tag="r")
                    nc.vector.reciprocal(r_sb[:], o_ps[:, BLOCK : BLOCK + 1])
                    # normalize: alternate engines (ACT gets the even blocks;
                    # at the tail the last block goes to DVE, which is faster
                    # and keeps the chain off the exp-busy ACT queue)
                    if j % 2 == 0:
                        nc.scalar.activation(
                            o_mt[:, j], o_ps[:, 0:BLOCK], Copy, scale=r_sb[:]
                        )
                    else:
                        nc.vector.tensor_scalar_mul(o_mt[:, j], o_ps[:, 0:BLOCK], r_sb[:])
                    if last:
                        # store per block as it completes, alternating rings
                        eng = nc.sync if j % 2 == 0 else nc.scalar
                        eng.dma_start(out_view[:, j : j + 1], o_mt[:, j : j + 1])

                if not last:
                    nc.gpsimd.dma_start(out_view, o_mt[:])

    nc.compile()
    return nc


def _get_nc():
    if "nc" not in _CACHE:
        _CACHE["nc"] = _build()
    return _CACHE["nc"]


def make_in_maps(x, Wq, Wk, Wv):
    import ml_dtypes

    proj_np = ml_dtypes.bfloat16
    # wp[p, i, k, d] = W_i[k*128 + p, d]
    wp = np.stack(
        [np.asarray(w).reshape(KC, 128, Dh).transpose(1, 0, 2) for w in (Wq, Wk, Wv)],
        axis=1,
    )
    wp_h = np.ascontiguousarray(wp.astype(proj_np))
    ident_h = np.eye(128, dtype=proj_np)
    x = np.asarray(x)
    maps = []
    for b in range(B):
        # xp[p, k, s] = x[b].T[k*128 + p, s]
        xp = np.asarray(x[b], dtype=proj_np).T.reshape(KC, 128, S).transpose(1, 0, 2)
        maps.append(
            {
                "xp": np.ascontiguousarray(xp),
                "wp": wp_h,
                "ident": ident_h,
            }
        )
    return maps


def kernel(x, Wq, Wk, Wv):
    from concourse.bass_utils import run_bass_kernel_spmd

    nc = _get_nc()
    in_maps = make_in_maps(x, Wq, Wk, Wv)
    res = run_bass_kernel_spmd(nc, in_maps, list(range(B))).results
    return np.stack([res[b]["out"] for b in range(B)], axis=0)
